# revision 13
# baseline (speedup 1.0000x reference)
"""Trainium2 Bass kernel for nn_GaussianSelfAttention (B=64, S=197, D=768).

Math: the reference's softmax is over a singleton axis, so attn == 1.0 exactly
and out = concat([ones(B,1,D), sample_v], axis=1) where
sample_v = (G @ x) @ Wv + wsum*bv,  G = per-image (196,197) bilinear one-hot
matrix built from Gaussian-sampled keys. q/k projections are dead code.

Device strategy (8 cores, data-parallel over batch, 8 images/core):
  - host builds per-image gather matrices from the tiny O(B*P) index math
  - the 4 bilinear tap indices are (14*py+px) mod 197 with the raw value
    confined to a ~115-wide band per image, so the mod-197 gather contracts
    over at most 128 distinct x rows: host packs those rows (a per-image
    rotation) into a single 128-partition chunk -> the gather is ONE
    matmul per (image, d-chunk) at N=196 (9.4k PE cycles total)
  - everything on the PE runs in fp16 (1 cycle/row)
  - projection: out^T[dout,q] = Wv^T @ sxT via matmul(lhsT=wv chunk,
    rhs=sxT, N=1568 split into 4x392 chunks) -> no ragged m-chunk,
    no transposes, no on-device index math (56.4k PE cycles)
  - PE does only matmuls (~66k cycles ~ 27.5us); DVE/Act do PSUM->SBUF
    fp16 copies; per-image input DMAs alternate the SP/Act HWDGE queues;
    Wv rides the gpsimd SWDGE queue
  - fallback (window > 128: never for N(0,1)-scale keys): generic 2-chunk
    contraction over all 197 rows
"""

import numpy as np

import concourse.bass as bass
import concourse.mybir as mybir
import concourse.tile as tile
from concourse import bacc, bass_utils

B, S, D, P = 64, 197, 768, 196
N_CORES = 8
BPC = B // N_CORES            # images per core
Q = BPC * P                   # 1568 sampled rows per core
GRID = 14.0
W2 = 2 * P                    # 392: projection q-chunk width

F16 = mybir.dt.float16
F32 = mybir.dt.float32

_NC = {}
_RUNNER = {}
_MODE = "win"  # "win" = single-chunk rotated-window gather, "gen" = fallback


def _emit(nc, iters=1, mode="win"):
    win = mode == "win"
    xw = (1 if win else 2) * BPC * D
    gw = BPC * (P if win else W2)
    x_d = nc.dram_tensor("x0", (128, xw), F16, kind="ExternalInput")
    wv_d = nc.dram_tensor("wv0", (128, 6 * D), F16, kind="ExternalInput")
    gt_d = nc.dram_tensor("gt0", (128, gw), F16, kind="ExternalInput")
    o_d = nc.dram_tensor("o0", (128, 6 * Q), F16, kind="ExternalOutput")

    with tile.TileContext(nc) as tc:
        with (
            tc.tile_pool(name="xb", bufs=2) as xpool,
            tc.tile_pool(name="wvp", bufs=2) as wpool,
            tc.tile_pool(name="gtp", bufs=2) as gtpool,
            tc.tile_pool(name="sxp", bufs=2) as spool,
            tc.tile_pool(name="ost", bufs=2) as opool,
            tc.tile_pool(name="psA", bufs=4, space="PSUM") as psA,
            tc.tile_pool(name="psB", bufs=3, space="PSUM") as psB,
        ):
            def body():
                xall = xpool.tile([128, xw], F16, name="xall", tag="xall")
                wvt = wpool.tile([128, 6 * D], F16, name="wvt", tag="wvt")
                gtall = gtpool.tile([128, gw], F16, name="gtall", tag="gtall")
                # per-image input DMAs, alternating SP / Act queues
                gp = gw // BPC
                xp = xw // BPC
                for b in range(BPC):
                    eng = nc.sync if b % 2 == 0 else nc.scalar
                    eng.dma_start(out=gtall[:, b * gp:(b + 1) * gp],
                                  in_=gt_d[:, b * gp:(b + 1) * gp])
                    eng.dma_start(out=xall[:, b * xp:(b + 1) * xp],
                                  in_=x_d[:, b * xp:(b + 1) * xp])
                nc.gpsimd.dma_start(out=wvt[:], in_=wv_d[:])

                sxT = [spool.tile([128, Q], F16, name=f"sxT{kc}",
                                  tag=f"sxT{kc}") for kc in range(6)]

                def gather(b):
                    for mj in range(6):
                        pa = psA.tile([128, P], F32, name="pa", tag="pa")
                        if win:
                            nc.tensor.matmul(
                                pa[:],
                                lhsT=xall[:, b * D + mj * 128:
                                          b * D + (mj + 1) * 128],
                                rhs=gtall[:, b * P:(b + 1) * P],
                                start=True, stop=True)
                        else:
                            for c, pn in ((0, 128), (1, 69)):
                                t = 2 * b + c
                                nc.tensor.matmul(
                                    pa[:],
                                    lhsT=xall[0:pn, t * D + mj * 128:
                                              t * D + (mj + 1) * 128],
                                    rhs=gtall[0:pn, b * W2 + c * P:
                                              b * W2 + (c + 1) * P],
                                    start=(c == 0), stop=(c == 1))
                        eng = nc.vector.tensor_copy if mj % 2 else nc.scalar.copy
                        eng(out=sxT[mj][:, b * P:(b + 1) * P], in_=pa[:])

                ot = [opool.tile([128, Q], F16, name=f"ot{m}", tag=f"ot{m}")
                      for m in range(6)]

                def proj(qn):
                    for m in range(6):
                        pb = psB.tile([128, W2], F32, name="pb", tag="pb")
                        for kc in range(6):
                            nc.tensor.matmul(
                                pb[:],
                                lhsT=wvt[:, kc * D + m * 128:
                                         kc * D + (m + 1) * 128],
                                rhs=sxT[kc][:, qn * W2:(qn + 1) * W2],
                                start=(kc == 0), stop=(kc == 5))
                        eng = nc.vector.tensor_copy if m % 2 else nc.scalar.copy
                        eng(out=ot[m][:, qn * W2:(qn + 1) * W2], in_=pb[:])

                for b in range(BPC):
                    gather(b)
                for qn in range(4):
                    proj(qn)
                for m in range(6):
                    nc.sync.dma_start(out=o_d[:, m * Q:(m + 1) * Q],
                                      in_=ot[m][:])

            if iters == 1:
                body()
            else:
                with tc.For_i(0, iters, 1):
                    body()


def _build(iters=1, mode="win"):
    key = (iters, mode)
    if key not in _NC:
        nc = bacc.Bacc("TRN2", target_bir_lowering=False, debug=False,
                       num_devices=N_CORES)
        _emit(nc, iters, mode)
        nc.compile()
        _NC[key] = nc
    return _NC[key]


def _sample_params(img_ids, avgs, std_devs, noise):
    """Per-(b,p) bilinear taps: raw grid values (pre-mod) and weights,
    replicating the reference's fp32 math (int32 truncation)."""
    ids = np.asarray(img_ids).astype(np.int64)
    a = np.asarray(avgs, np.float32)[ids]        # (B,2,P)
    s = np.asarray(std_devs, np.float32)[ids]
    nz = np.asarray(noise, np.float32)
    kx = (nz[:, 0] - a[:, 0]) / s[:, 0]          # (B,P) f32
    ky = (nz[:, 1] - a[:, 1]) / s[:, 1]
    x1, x2 = np.ceil(kx), np.floor(kx)
    y1, y2 = np.ceil(ky), np.floor(ky)
    wx1, wx2 = 1.0 - np.abs(x1 - kx), 1.0 - np.abs(x2 - kx)
    wy1, wy2 = 1.0 - np.abs(y1 - ky), 1.0 - np.abs(y2 - ky)
    taps = []
    for px, wx in ((x1, wx1), (x2, wx2)):
        for py, wy in ((y1, wy1), (y2, wy2)):
            raw = (np.float32(GRID) * py + px).astype(np.int32)
            taps.append((raw, (wx * wy).astype(np.float32)))
    return taps


def _pack_inputs(x, img_ids, Wv, avgs, std_devs, noise):
    global _MODE
    x = np.asarray(x, np.float32)
    wv = np.asarray(Wv, np.float32)
    wvp = np.ascontiguousarray(
        wv.reshape(6, 128, D).transpose(1, 0, 2).reshape(128, 6 * D)
    ).astype(np.float16)

    taps = _sample_params(img_ids, avgs, std_devs, noise)
    raws = np.stack([t[0] for t in taps])        # (4,B,P) pre-mod grid values
    rlo = raws.min(axis=(0, 2))                  # (B,)
    wid = raws.max(axis=(0, 2)) - rlo + 1
    _MODE = "win" if int(wid.max()) <= 128 else "gen"
    bidx = np.arange(B)[:, None]
    pidx = np.arange(P)[None, :]

    if _MODE == "win":
        # gather matrix in rotated-window row space: row k <-> x row
        # (rlo[b] + k) mod 197
        G = np.zeros((B, 128, P), np.float32)
        for raw, w in taps:
            np.add.at(G, (bidx, raw - rlo[:, None], pidx), w)
        G16 = G.astype(np.float16)
        rows = (rlo[:, None] + np.arange(128)[None, :]) % S    # (B,128)
        xw = np.take_along_axis(
            x, rows[:, :, None], axis=1).astype(np.float16)    # (B,128,D)
        in_maps = []
        for c in range(N_CORES):
            sl = slice(c * BPC, (c + 1) * BPC)
            xp = np.ascontiguousarray(
                xw[sl].transpose(1, 0, 2).reshape(128, BPC * D))
            gpk = np.ascontiguousarray(
                G16[sl].transpose(1, 0, 2).reshape(128, BPC * P))
            in_maps.append({"x0": xp, "wv0": wvp, "gt0": gpk})
        return in_maps

    # generic fallback: contract over all 197 rows in two chunks
    G = np.zeros((B, S, P), np.float32)
    for raw, w in taps:
        np.add.at(G, (bidx, raw % S, pidx), w)
    G16 = G.astype(np.float16)
    in_maps = []
    for c in range(N_CORES):
        xt = np.zeros((2 * BPC, 128, D), np.float16)
        gpk = np.zeros((128, BPC * W2), np.float16)
        for b in range(BPC):
            im = c * BPC + b
            xt[2 * b] = x[im, 0:128]
            xt[2 * b + 1, :69] = x[im, 128:S]
            gpk[:, b * W2:b * W2 + P] = G16[im, 0:128]
            gpk[0:69, b * W2 + P:(b + 1) * W2] = G16[im, 128:S]
        xp = np.ascontiguousarray(
            xt.transpose(1, 0, 2).reshape(128, 2 * BPC * D))
        in_maps.append({"x0": xp, "wv0": wvp, "gt0": gpk})
    return in_maps


def _unpack_out(o_np):
    # o_np: (128, 6*Q) fp16 = out^T chunks -> (BPC, S, D) with ones rows
    svT = o_np.reshape(128, 6, Q).transpose(1, 0, 2).reshape(D, Q)
    out = np.ones((BPC, S, D), np.float32)
    out[:, 1:, :] = svT.T.astype(np.float32).reshape(BPC, P, D)
    return out


def _get_runner(iters=1):
    """Build the sharded PJRT callable once and cache it."""
    key = (iters, _MODE)
    if key in _RUNNER:
        return _RUNNER[key]
    import jax
    from jax.experimental.shard_map import shard_map
    from jax.sharding import Mesh, PartitionSpec
    from concourse import bass2jax, mybir as _mybir

    nc = _build(iters, _MODE)
    bass2jax.install_neuronx_cc_hook()
    in_names, out_names, out_avals, zero_outs = [], [], [], []
    part_name = (nc.partition_id_tensor.name
                 if nc.partition_id_tensor else None)
    for alloc in nc.m.functions[0].allocations:
        if not isinstance(alloc, _mybir.MemoryLocationSet):
            continue
        name = alloc.memorylocations[0].name
        if alloc.kind == "ExternalInput":
            if name != part_name:
                in_names.append(name)
        elif alloc.kind == "ExternalOutput":
            shape = tuple(alloc.tensor_shape)
            dtype = _mybir.dt.np(alloc.dtype)
            out_names.append(name)
            out_avals.append(jax.core.ShapedArray(shape, dtype))
            zero_outs.append(np.zeros(shape, dtype))
    n_params = len(in_names)
    all_names = in_names + out_names
    if part_name is not None:
        all_names = all_names + [part_name]
    donate = tuple(range(n_params, n_params + len(out_names)))

    def _body(*args):
        operands = list(args)
        if part_name is not None:
            operands.append(bass2jax.partition_id_tensor())
        outs = bass2jax._bass_exec_p.bind(
            *operands,
            out_avals=tuple(out_avals),
            in_names=tuple(all_names),
            out_names=tuple(out_names),
            lowering_input_output_aliases=(),
            sim_require_finite=True,
            sim_require_nnan=True,
            nc=nc,
        )
        return tuple(outs)

    devices = jax.devices()[:N_CORES]
    mesh = Mesh(np.asarray(devices), ("core",))
    specs = (PartitionSpec("core"),) * (n_params + len(out_names))
    fn = jax.jit(
        shard_map(_body, mesh=mesh, in_specs=specs,
                  out_specs=(PartitionSpec("core"),) * len(out_names),
                  check_rep=False),
        donate_argnums=donate, keep_unused=True)

    def run(in_maps):
        concat_in = [
            np.concatenate([np.asarray(m[nm]) for m in in_maps], axis=0)
            for nm in in_names
        ]
        concat_zero = [
            np.zeros((N_CORES * z.shape[0], *z.shape[1:]), z.dtype)
            for z in zero_outs
        ]
        arrs = fn(*concat_in, *concat_zero)
        return [
            {nm: np.asarray(arrs[i]).reshape(N_CORES, *out_avals[i].shape)[c]
             for i, nm in enumerate(out_names)}
            for c in range(N_CORES)
        ]

    _RUNNER[key] = run
    return run


class _Res:
    def __init__(self, results):
        self.results = results
        self.exec_time_ns = None


def run_cores(in_maps, trace=False, iters=1):
    return _Res(_get_runner(iters)(in_maps))


def kernel(x, img_ids, mask=None, Wq=None, bq=None, Wk=None, bk=None,
           Wv=None, bv=None, avgs=None, std_devs=None, noise=None,
           _trace=False, _results=None):
    in_maps = _pack_inputs(x, img_ids, Wv, avgs, std_devs, noise)
    res = run_cores(in_maps, trace=_trace)
    if _results is not None:
        _results.append(res)
    out = np.concatenate(
        [_unpack_out(res.results[c]["o0"]) for c in range(N_CORES)], axis=0)
    bv_np = np.asarray(bv, np.float32) if bv is not None else None
    if bv_np is not None and np.any(bv_np):
        # sample() is affine: add (sum_i w_i) * bv for the sampled rows.
        wsum = np.zeros((B, P), np.float32)
        for _, w in _sample_params(img_ids, avgs, std_devs, noise):
            wsum += w
        out[:, 1:, :] += wsum[:, :, None] * bv_np[None, None, :]
    return out


# revision 26
# speedup vs baseline: 1.9062x; 1.9062x over previous
"""Trainium2 Bass kernel for nn_GaussianSelfAttention (B=64, S=197, D=768).

Math: the reference's softmax is over a singleton axis, so attn == 1.0 exactly
and out = concat([ones(B,1,D), sample_v], axis=1) where
sample_v = (G @ x) @ Wv + wsum*bv,  G = per-image (196,197) bilinear one-hot
matrix built from Gaussian-sampled keys. q/k projections are dead code.

Device strategy (8 cores, data-parallel over batch, 8 images/core):
  - host builds per-image gather matrices from the tiny O(B*P) index math
  - the 4 bilinear tap indices are (14*py+px) mod 197 with the raw value
    confined to a ~115-wide band per image, so the mod-197 gather contracts
    over at most 128 distinct x rows: host packs those rows (a per-image
    rotation) into a single 128-partition chunk -> the gather is ONE
    matmul per (image, d-chunk) at N=196 (9.4k PE cycles total)
  - everything on the PE runs in fp16 (1 cycle/row)
  - projection: out^T[dout,q] = Wv^T @ sxT via matmul(lhsT=wv chunk,
    rhs=sxT, N=1568 split into 4x392 chunks) -> no ragged m-chunk,
    no transposes, no on-device index math (56.4k PE cycles)
  - PE does only matmuls (~66k cycles ~ 27.5us); DVE/Act do PSUM->SBUF
    fp16 copies; per-image input DMAs alternate the SP/Act HWDGE queues;
    Wv rides the gpsimd SWDGE queue
  - fallback (window > 128: never for N(0,1)-scale keys): generic 2-chunk
    contraction over all 197 rows
"""

import numpy as np

import concourse.bass as bass
import concourse.mybir as mybir
import concourse.tile as tile
from concourse import bacc, bass_utils

B, S, D, P = 64, 197, 768, 196
N_CORES = 8
BPC = B // N_CORES            # images per core
Q = BPC * P                   # 1568 sampled rows per core
GRID = 14.0
W2 = 2 * P                    # 392: projection q-chunk width

F16 = mybir.dt.float16
F32 = mybir.dt.float32

_NC = {}
_RUNNER = {}
_MODE = "win"  # "win" = single-chunk rotated-window gather, "gen" = fallback
PSA, PSB = 4, 4
_SCHED = tuple(
    [("g", 0), ("g", 1), ("g", 2), ("g", 3), ("g", 4), ("g", 5),
     ("p", 0), ("p", 1), ("g", 6), ("g", 7), ("p", 2), ("p", 3)])


def _emit(nc, iters=1, mode="win", loop=True):
    win = mode == "win"
    xw = (1 if win else 2) * BPC * D
    gw = BPC * (P if win else W2)
    x_d = nc.dram_tensor("x0", (128, xw), F16, kind="ExternalInput")
    wv_d = nc.dram_tensor("wv0", (128, 6 * D), F16, kind="ExternalInput")
    gt_d = nc.dram_tensor("gt0", (128, gw), F16, kind="ExternalInput")
    o_d = nc.dram_tensor("o0", (128, 6, Q), F16, kind="ExternalOutput")

    with tile.TileContext(nc) as tc:
        with (
            tc.tile_pool(name="xb", bufs=2) as xpool,
            tc.tile_pool(name="wvp", bufs=2) as wpool,
            tc.tile_pool(name="gtp", bufs=2) as gtpool,
            tc.tile_pool(name="sxp", bufs=2) as spool,
            tc.tile_pool(name="ost", bufs=2) as opool,
            tc.tile_pool(name="psA", bufs=PSA, space="PSUM") as psA,
            tc.tile_pool(name="psB", bufs=PSB, space="PSUM") as psB,
        ):
            def body():
                xall = xpool.tile([128, xw], F16, name="xall", tag="xall")
                wvt = wpool.tile([128, 6 * D], F16, name="wvt", tag="wvt")
                gtall = gtpool.tile([128, gw], F16, name="gtall", tag="gtall")
                # per-image input DMAs, alternating SP / Act queues
                gp = gw // BPC
                xp = xw // BPC
                for b in range(BPC):
                    eng = nc.sync if b % 2 == 0 else nc.scalar
                    eng.dma_start(out=gtall[:, b * gp:(b + 1) * gp],
                                  in_=gt_d[:, b * gp:(b + 1) * gp])
                    eng.dma_start(out=xall[:, b * xp:(b + 1) * xp],
                                  in_=x_d[:, b * xp:(b + 1) * xp])
                nc.gpsimd.dma_start(out=wvt[:], in_=wv_d[:])

                sxT = spool.tile([128, 6, Q], F16, name="sxT", tag="sxT")

                def gather(b):
                    for j in range(3):
                        pa = psA.tile([128, 2, P], F32, name="pa", tag="pa")
                        for h in range(2):
                            mj = 2 * j + h
                            if win:
                                nc.tensor.matmul(
                                    pa[:, h],
                                    lhsT=xall[:, b * D + mj * 128:
                                              b * D + (mj + 1) * 128],
                                    rhs=gtall[:, b * P:(b + 1) * P],
                                    start=True, stop=True)
                            else:
                                for c, pn in ((0, 128), (1, 69)):
                                    t = 2 * b + c
                                    nc.tensor.matmul(
                                        pa[:, h],
                                        lhsT=xall[0:pn, t * D + mj * 128:
                                                  t * D + (mj + 1) * 128],
                                        rhs=gtall[0:pn, b * W2 + c * P:
                                                  b * W2 + (c + 1) * P],
                                        start=(c == 0), stop=(c == 1))
                        eng = (nc.vector.tensor_copy if (3 * b + j) % 2
                               else nc.scalar.copy)
                        eng(out=sxT[:, 2 * j:2 * j + 2, b * P:(b + 1) * P],
                            in_=pa[:])

                def proj(qn):
                    otq = opool.tile([128, 6, W2], F16, name="otq", tag="otq")
                    for m in range(6):
                        pb = psB.tile([128, W2], F32, name="pb", tag="pb")
                        for kc in range(6):
                            nc.tensor.matmul(
                                pb[:],
                                lhsT=wvt[:, kc * D + m * 128:
                                         kc * D + (m + 1) * 128],
                                rhs=sxT[:, kc, qn * W2:(qn + 1) * W2],
                                start=(kc == 0), stop=(kc == 5))
                        eng = nc.vector.tensor_copy if m % 2 else nc.scalar.copy
                        eng(out=otq[:, m, :], in_=pb[:])
                    # early per-qn output flush; keep the Act queue input-only
                    oeng = nc.sync if qn % 2 == 0 else nc.gpsimd
                    oeng.dma_start(out=o_d[:, :, qn * W2:(qn + 1) * W2],
                                   in_=otq[:])

                # PE order: gathers feeding interleaved projections
                for op, i in _SCHED:
                    (gather if op == "g" else proj)(i)

            if iters == 1:
                body()
            elif not loop:
                for _ in range(iters):
                    body()
            else:
                with tc.For_i(0, iters, 1):
                    body()


def _build(iters=1, mode="win", loop=True):
    key = (iters, mode, loop, _SCHED, PSA, PSB)
    if key not in _NC:
        nc = bacc.Bacc("TRN2", target_bir_lowering=False, debug=False,
                       num_devices=N_CORES)
        _emit(nc, iters, mode, loop)
        nc.compile()
        _NC[key] = nc
    return _NC[key]


def _sample_params(img_ids, avgs, std_devs, noise):
    """Per-(b,p) bilinear taps: raw grid values (pre-mod) and weights,
    replicating the reference's fp32 math (int32 truncation)."""
    ids = np.asarray(img_ids).astype(np.int64)
    a = np.asarray(avgs, np.float32)[ids]        # (B,2,P)
    s = np.asarray(std_devs, np.float32)[ids]
    nz = np.asarray(noise, np.float32)
    kx = (nz[:, 0] - a[:, 0]) / s[:, 0]          # (B,P) f32
    ky = (nz[:, 1] - a[:, 1]) / s[:, 1]
    x1, x2 = np.ceil(kx), np.floor(kx)
    y1, y2 = np.ceil(ky), np.floor(ky)
    wx1, wx2 = 1.0 - np.abs(x1 - kx), 1.0 - np.abs(x2 - kx)
    wy1, wy2 = 1.0 - np.abs(y1 - ky), 1.0 - np.abs(y2 - ky)
    taps = []
    for px, wx in ((x1, wx1), (x2, wx2)):
        for py, wy in ((y1, wy1), (y2, wy2)):
            raw = (np.float32(GRID) * py + px).astype(np.int32)
            taps.append((raw, (wx * wy).astype(np.float32)))
    return taps


def _pack_inputs(x, img_ids, Wv, avgs, std_devs, noise):
    global _MODE
    x = np.asarray(x, np.float32)
    wv = np.asarray(Wv, np.float32)
    wvp = np.ascontiguousarray(
        wv.reshape(6, 128, D).transpose(1, 0, 2).reshape(128, 6 * D)
    ).astype(np.float16)

    taps = _sample_params(img_ids, avgs, std_devs, noise)
    raws = np.stack([t[0] for t in taps])        # (4,B,P) pre-mod grid values
    rlo = raws.min(axis=(0, 2))                  # (B,)
    wid = raws.max(axis=(0, 2)) - rlo + 1
    _MODE = "win" if int(wid.max()) <= 128 else "gen"
    bidx = np.arange(B)[:, None]
    pidx = np.arange(P)[None, :]

    if _MODE == "win":
        # gather matrix in rotated-window row space: row k <-> x row
        # (rlo[b] + k) mod 197
        G = np.zeros((B, 128, P), np.float32)
        for raw, w in taps:
            np.add.at(G, (bidx, raw - rlo[:, None], pidx), w)
        G16 = G.astype(np.float16)
        rows = (rlo[:, None] + np.arange(128)[None, :]) % S    # (B,128)
        xw = np.take_along_axis(
            x, rows[:, :, None], axis=1).astype(np.float16)    # (B,128,D)
        in_maps = []
        for c in range(N_CORES):
            sl = slice(c * BPC, (c + 1) * BPC)
            xp = np.ascontiguousarray(
                xw[sl].transpose(1, 0, 2).reshape(128, BPC * D))
            gpk = np.ascontiguousarray(
                G16[sl].transpose(1, 0, 2).reshape(128, BPC * P))
            in_maps.append({"x0": xp, "wv0": wvp, "gt0": gpk})
        return in_maps

    # generic fallback: contract over all 197 rows in two chunks
    G = np.zeros((B, S, P), np.float32)
    for raw, w in taps:
        np.add.at(G, (bidx, raw % S, pidx), w)
    G16 = G.astype(np.float16)
    in_maps = []
    for c in range(N_CORES):
        xt = np.zeros((2 * BPC, 128, D), np.float16)
        gpk = np.zeros((128, BPC * W2), np.float16)
        for b in range(BPC):
            im = c * BPC + b
            xt[2 * b] = x[im, 0:128]
            xt[2 * b + 1, :69] = x[im, 128:S]
            gpk[:, b * W2:b * W2 + P] = G16[im, 0:128]
            gpk[0:69, b * W2 + P:(b + 1) * W2] = G16[im, 128:S]
        xp = np.ascontiguousarray(
            xt.transpose(1, 0, 2).reshape(128, 2 * BPC * D))
        in_maps.append({"x0": xp, "wv0": wvp, "gt0": gpk})
    return in_maps


def _unpack_out(o_np):
    # o_np: (128, 6, Q) fp16 = out^T chunks -> (BPC, S, D) with ones rows
    svT = o_np.reshape(128, 6, Q).transpose(1, 0, 2).reshape(D, Q)
    out = np.ones((BPC, S, D), np.float32)
    out[:, 1:, :] = svT.T.astype(np.float32).reshape(BPC, P, D)
    return out


def _get_runner(iters=1):
    """Build the sharded PJRT callable once and cache it."""
    key = (iters, _MODE)
    if key in _RUNNER:
        return _RUNNER[key]
    import jax
    from jax.experimental.shard_map import shard_map
    from jax.sharding import Mesh, PartitionSpec
    from concourse import bass2jax, mybir as _mybir

    nc = _build(iters, _MODE, loop=True)
    bass2jax.install_neuronx_cc_hook()
    in_names, out_names, out_avals, zero_outs = [], [], [], []
    part_name = (nc.partition_id_tensor.name
                 if nc.partition_id_tensor else None)
    for alloc in nc.m.functions[0].allocations:
        if not isinstance(alloc, _mybir.MemoryLocationSet):
            continue
        name = alloc.memorylocations[0].name
        if alloc.kind == "ExternalInput":
            if name != part_name:
                in_names.append(name)
        elif alloc.kind == "ExternalOutput":
            shape = tuple(alloc.tensor_shape)
            dtype = _mybir.dt.np(alloc.dtype)
            out_names.append(name)
            out_avals.append(jax.core.ShapedArray(shape, dtype))
            zero_outs.append(np.zeros(shape, dtype))
    n_params = len(in_names)
    all_names = in_names + out_names
    if part_name is not None:
        all_names = all_names + [part_name]
    donate = tuple(range(n_params, n_params + len(out_names)))

    def _body(*args):
        operands = list(args)
        if part_name is not None:
            operands.append(bass2jax.partition_id_tensor())
        outs = bass2jax._bass_exec_p.bind(
            *operands,
            out_avals=tuple(out_avals),
            in_names=tuple(all_names),
            out_names=tuple(out_names),
            lowering_input_output_aliases=(),
            sim_require_finite=True,
            sim_require_nnan=True,
            nc=nc,
        )
        return tuple(outs)

    devices = jax.devices()[:N_CORES]
    mesh = Mesh(np.asarray(devices), ("core",))
    specs = (PartitionSpec("core"),) * (n_params + len(out_names))
    fn = jax.jit(
        shard_map(_body, mesh=mesh, in_specs=specs,
                  out_specs=(PartitionSpec("core"),) * len(out_names),
                  check_rep=False),
        donate_argnums=donate, keep_unused=True)

    def run(in_maps):
        concat_in = [
            np.concatenate([np.asarray(m[nm]) for m in in_maps], axis=0)
            for nm in in_names
        ]
        concat_zero = [
            np.zeros((N_CORES * z.shape[0], *z.shape[1:]), z.dtype)
            for z in zero_outs
        ]
        arrs = fn(*concat_in, *concat_zero)
        return [
            {nm: np.asarray(arrs[i]).reshape(N_CORES, *out_avals[i].shape)[c]
             for i, nm in enumerate(out_names)}
            for c in range(N_CORES)
        ]

    _RUNNER[key] = run
    return run


class _Res:
    def __init__(self, results):
        self.results = results
        self.exec_time_ns = None


def run_cores(in_maps, trace=False, iters=1):
    return _Res(_get_runner(iters)(in_maps))


def kernel(x, img_ids, mask=None, Wq=None, bq=None, Wk=None, bk=None,
           Wv=None, bv=None, avgs=None, std_devs=None, noise=None,
           _trace=False, _results=None):
    in_maps = _pack_inputs(x, img_ids, Wv, avgs, std_devs, noise)
    res = run_cores(in_maps, trace=_trace)
    if _results is not None:
        _results.append(res)
    out = np.concatenate(
        [_unpack_out(res.results[c]["o0"]) for c in range(N_CORES)], axis=0)
    bv_np = np.asarray(bv, np.float32) if bv is not None else None
    if bv_np is not None and np.any(bv_np):
        # sample() is affine: add (sum_i w_i) * bv for the sampled rows.
        wsum = np.zeros((B, P), np.float32)
        for _, w in _sample_params(img_ids, avgs, std_devs, noise):
            wsum += w
        out[:, 1:, :] += wsum[:, :, None] * bv_np[None, None, :]
    return out
